# revision 6
# baseline (speedup 1.0000x reference)
"""HAN layer (3-metapath GAT + semantic attention) on 8 TRN2 NeuronCores.

Sharding: nodes partitioned 6250/core; edges sharded by dst-node owner.
Each core projects only its LOCAL nodes (T_local = h_local @ Wp giving
[er|el|feat64] per metapath, paths interleaved so the full table row of
(node n, path p) is 3n+p), then an on-device AllGather builds the full
50k-node table on every core (~5x less host->device traffic than
shipping the full table per core). Per-core dst nodes are processed one
node per SBUF partition lane (degree-sorted for load balance); edges are
gathered per round with indirect DMA from the all-gathered table, the
metapath selected via the DMA element_offset. Padding uses sentinel
table rows (el=-300 => exp ~ 0, feat=0) so no masks are needed.
Aggregation runs on the vector engine (multiply + strided reduce), not
per-edge matmuls. Semantic attention uses a tiny AllReduce.

Host-side: the per-tile round schedule (BV) is deterministic for the
fixed problem instance, so the bass build + walrus compile + PJRT
executable (and the donated output buffers) are all prepared at import
time; kernel() itself only does numpy preprocessing, async device_put
(dispatched as soon as each array is ready) and the execution.
"""

import time as _time

import numpy as np
import ml_dtypes
import jax
import jax.numpy as jnp
from jax.sharding import Mesh, PartitionSpec, NamedSharding
from jax.experimental.shard_map import shard_map

import concourse.bass as bass
import concourse.tile as tile
import concourse.bass2jax as b2j
from concourse import bacc, mybir
from concourse.masks import make_identity

N = 50000
E = 800000
P = 3
IN = 256
D = 64
SEM_H = 128
NEG = 0.2
NC_ = 8
NSH = N // NC_            # 6250 nodes per core
NT = (NSH + 127) // 128   # 49 node tiles per core
NTL = NT * 128            # padded lanes per core
SENTN = N                 # sentinel node id (pad); table rows 3N..3N+2
RT = 3 * N + 8            # gathered table rows (3 sentinel rows + pad)
BF16 = mybir.dt.bfloat16
F32 = mybir.dt.float32
I32 = mybir.dt.int32
U16 = mybir.dt.uint16

# Per-tile gather round counts for the fixed problem instance
# (seed-0 edge lists; max over 8 cores and 3 paths of the per-128-lane
# incoming-degree maximum, lanes degree-sorted). Recomputed at runtime;
# if it ever differs the kernel is rebuilt on the fly.
BV_DEFAULT = [36, 34, 34, 32, 32, 33, 30, 32, 30, 30, 30, 29, 29, 29,
              29, 28, 28, 29, 28, 30, 30, 28, 27, 28, 27, 27, 27, 29,
              26, 28, 28, 26, 25, 28, 25, 26, 25, 25, 26, 26, 25, 25,
              27, 23, 25, 24, 24, 22, 19]

LAST_WALL_NS = 0.0
STAGES = {}

_DEVICES = jax.devices()[:NC_]
_MESH = Mesh(np.asarray(_DEVICES), ("core",))
_SHARD = NamedSharding(_MESH, PartitionSpec("core"))
try:
    _WARM = jax.device_put(np.zeros((NC_, 8), np.float32), _SHARD)
except Exception:
    _WARM = None


def _build(Bv):
    t0 = _time.perf_counter()
    Bmax = max(Bv)
    CT = int(sum(Bv)) * P
    nc = bacc.Bacc("TRN2", target_bir_lowering=False, debug=False)
    hTk = nc.dram_tensor("hTk", [IN, NSH], BF16, kind="ExternalInput").ap()
    # WpS rows 0..255: fused projection weights [er_w|el_w|feat_w] per
    # path; rows 256..258: the sentinel table rows (cols 0..65).
    WpS = nc.dram_tensor("WpS", [IN + P, P * 66], BF16,
                         kind="ExternalInput").ap()
    srcM = nc.dram_tensor("srcM", [128, CT], U16, kind="ExternalInput").ap()
    # IDX cols [0,NT): scatter row ids; [NT,2NT): lane validity 0/1;
    # [2NT,3NT): global node id of each lane (for the er gather).
    IDX = nc.dram_tensor("IDX", [128, 3 * NT], I32,
                         kind="ExternalInput").ap()
    # WS cols 0..127: sem_W1 (rows 0..63); col 128: sem_b1; col 129: sem_w2
    WS = nc.dram_tensor("WS", [SEM_H, SEM_H + 2], F32,
                        kind="ExternalInput").ap()
    out = nc.dram_tensor("out", [NSH, D], BF16, kind="ExternalOutput").ap()
    Tloc = nc.dram_tensor("Tloc", [NSH, P, 66], BF16).ap()
    Tg = nc.dram_tensor("Tg", [RT, 66], BF16, addr_space="Shared").ap()
    crin = nc.dram_tensor("crin", [1, 4], F32).ap()
    crout = nc.dram_tensor("crout", [1, 4], F32, addr_space="Shared").ap()

    with tile.TileContext(nc) as tc:
        with (
            tc.tile_pool(name="persist", bufs=1) as pp,
            tc.tile_pool(name="work", bufs=3) as wp,
            tc.tile_pool(name="gpool", bufs=2) as gp,
            tc.tile_pool(name="psA", bufs=2, space="PSUM") as psa,
            tc.tile_pool(name="psS", bufs=1, space="PSUM") as ps1,
        ):
            Wp0 = pp.tile([128, P * 66], BF16)
            Wp1 = pp.tile([128, P * 66], BF16)
            nc.sync.dma_start(Wp0[:], WpS[0:128, :])
            nc.sync.dma_start(Wp1[:], WpS[128:256, :])
            sl = pp.tile([P, 66], BF16)
            nc.sync.dma_start(sl[:], WpS[256:256 + P, 0:66])
            identF = pp.tile([128, 128], F32)
            make_identity(nc, identF[:])
            WSt = pp.tile([SEM_H, SEM_H + 2], F32)
            nc.sync.dma_start(WSt[:], WS[:])
            W1sb = WSt[0:D, 0:SEM_H]
            b1sb = WSt[:, SEM_H:SEM_H + 1]
            w2sb = WSt[:, SEM_H + 1:SEM_H + 2]
            idx_t = pp.tile([128, 3 * NT], I32)
            nc.sync.dma_start(idx_t[:], IDX[:])
            row_t = idx_t[:, 0:NT]
            nmsk_t = pp.tile([128, NT], F32)
            nc.vector.tensor_copy(nmsk_t[:], idx_t[:, NT:2 * NT])
            gid3 = pp.tile([128, NT], I32)
            nc.vector.tensor_scalar_mul(gid3[:], idx_t[:, 2 * NT:3 * NT], 3)
            zbuf = pp.tile([128, NT, P, D], F32)
            wbuf = pp.tile([128, P * NT], F32)
            onesc = pp.tile([128, 1], F32)
            nc.gpsimd.memset(onesc[:], 1.0)
            ones1 = pp.tile([1, 128], F32)
            nc.gpsimd.memset(ones1[:], 1.0)
            # edge gather rows: si_all = 3 * src_node_id (path selected
            # via the gather's element_offset)
            siu = wp.tile([128, CT], U16, tag="siu")
            nc.sync.dma_start(siu[:], srcM[:])
            si_all = pp.tile([128, CT], I32)
            nc.vector.tensor_copy(si_all[:], siu[:])
            nc.vector.tensor_scalar_mul(si_all[:], si_all[:], 3)

            # ---- Phase A: T_local = h_localT.T @ Wp  (49 tiles) ----
            GRP = 8
            t0_ = 0
            while t0_ < NT:
                gt = min(GRP, NT - t0_)
                ncols = min(NSH - t0_ * 128, gt * 128)
                h0 = wp.tile([128, GRP * 128], BF16, tag="h0")
                h1 = wp.tile([128, GRP * 128], BF16, tag="h1")
                nc.sync.dma_start(h0[:, :ncols],
                                  hTk[0:128, t0_ * 128:t0_ * 128 + ncols])
                nc.sync.dma_start(h1[:, :ncols],
                                  hTk[128:256, t0_ * 128:t0_ * 128 + ncols])
                for i in range(gt):
                    t = t0_ + i
                    w = min(128, NSH - t * 128)
                    pa = psa.tile([128, P * 66], F32, tag="pa")
                    nc.tensor.matmul(out=pa[:w, :],
                                     lhsT=h0[:, i * 128:i * 128 + w],
                                     rhs=Wp0[:], start=True, stop=False)
                    nc.tensor.matmul(out=pa[:w, :],
                                     lhsT=h1[:, i * 128:i * 128 + w],
                                     rhs=Wp1[:], start=False, stop=True)
                    stg = wp.tile([128, P * 66], BF16, tag="stg")
                    nc.vector.tensor_copy(stg[:w, :], pa[:w, :])
                    for p in range(P):
                        nc.sync.dma_start(
                            Tloc[t * 128:t * 128 + w, p, :],
                            stg[:w, p * 66:(p + 1) * 66])
                t0_ += gt

            # ---- AllGather the table; sentinel rows after it ----
            nc.gpsimd.collective_compute(
                "AllGather", mybir.AluOpType.bypass,
                replica_groups=[list(range(NC_))],
                ins=[Tloc[:]], outs=[Tg[0:3 * N, :]])
            nc.sync.dma_start(Tg[3 * N:3 * N + P, :], sl[:])

            # ---- Phase B: per node tile, 3 paths stacked ----
            coff = np.cumsum([0] + [P * b for b in Bv])
            for v in range(NT):
                B = Bv[v]
                c0 = int(coff[v])
                G = gp.tile([128, P, Bmax, 66], BF16, tag="G")
                for p in range(P):
                    for b in range(B):
                        c = c0 + p * B + b
                        nc.gpsimd.indirect_dma_start(
                            out=G[:, p, b, :], out_offset=None, in_=Tg[:],
                            in_offset=bass.IndirectOffsetOnAxis(
                                ap=si_all[:, c:c + 1], axis=0),
                            element_offset=p * 66)
                er3 = wp.tile([128, P], BF16, tag="er3")
                for p in range(P):
                    nc.gpsimd.indirect_dma_start(
                        out=er3[:, p:p + 1], out_offset=None, in_=Tg[:],
                        in_offset=bass.IndirectOffsetOnAxis(
                            ap=gid3[:, v:v + 1], axis=0),
                        element_offset=p * 66)
                # e = leaky(el + er); ex = exp(e)
                Ef = wp.tile([128, P, Bmax], F32, tag="Ef")
                nc.vector.tensor_tensor(
                    out=Ef[:, :, :B], in0=G[:, :, :B, 1],
                    in1=er3[:, :, None].broadcast_to([128, P, B]),
                    op=mybir.AluOpType.add)
                Lk = wp.tile([128, P, Bmax], F32, tag="Lk")
                nc.vector.tensor_scalar_mul(Lk[:, :, :B], Ef[:, :, :B], NEG)
                nc.vector.tensor_tensor(out=Ef[:, :, :B], in0=Ef[:, :, :B],
                                        in1=Lk[:, :, :B],
                                        op=mybir.AluOpType.max)
                EX = wp.tile([128, P, Bmax], BF16, tag="EX")
                nc.scalar.activation(EX[:, :, :B], Ef[:, :, :B],
                                     mybir.ActivationFunctionType.Exp)
                den = wp.tile([128, P], F32, tag="den")
                nc.vector.reduce_sum(den[:, :, None], EX[:, :, :B],
                                     axis=mybir.AxisListType.X)
                # weighted aggregation on DVE
                FW = gp.tile([128, P, Bmax, D], BF16, tag="FW")
                nc.vector.tensor_tensor(
                    out=FW[:, :, :B, :], in0=G[:, :, :B, 2:66],
                    in1=EX[:, :, :B, None].broadcast_to([128, P, B, D]),
                    op=mybir.AluOpType.mult)
                agg = wp.tile([128, P, D], F32, tag="agg")
                nc.vector.reduce_sum(
                    agg[:, :, :, None],
                    FW[:, :, :B, :].rearrange("p q b d -> p q d b"),
                    axis=mybir.AxisListType.X)
                nc.vector.tensor_scalar_max(den[:], den[:], 1e-9)
                rec = wp.tile([128, P], F32, tag="rec")
                nc.vector.reciprocal(rec[:], den[:])
                zt = wp.tile([128, P, D], F32, tag="zt")
                nc.vector.tensor_tensor(
                    out=zt[:], in0=agg[:],
                    in1=rec[:, :, None].broadcast_to([128, P, D]),
                    op=mybir.AluOpType.mult)
                # elu: max(x,0) + exp(min(x,0)) - 1
                t1 = wp.tile([128, P, D], F32, tag="t1")
                nc.vector.tensor_scalar_min(t1[:], zt[:], 0.0)
                t2 = wp.tile([128, P, D], F32, tag="t2")
                nc.scalar.activation(t2[:], t1[:],
                                     mybir.ActivationFunctionType.Exp)
                t3 = wp.tile([128, P, D], F32, tag="t3")
                nc.vector.tensor_scalar_max(t3[:], zt[:], 0.0)
                nc.vector.tensor_tensor(out=t2[:], in0=t2[:], in1=t3[:],
                                        op=mybir.AluOpType.add)
                nc.vector.tensor_scalar_add(zbuf[:, v, :, :], t2[:], -1.0)
                # semantic score w = tanh(z @ W1 + b1) @ w2 per path
                ztT3 = wp.tile([D, P * 128], F32, tag="ztT3")
                for p in range(P):
                    pt = ps1.tile([D, 128], F32, tag="ps_t")
                    nc.tensor.transpose(out=pt[:], in_=zbuf[:, v, p, :],
                                        identity=identF[:])
                    nc.vector.tensor_copy(ztT3[:, p * 128:(p + 1) * 128],
                                          pt[:])
                ph = ps1.tile([SEM_H, P * 128], F32, tag="ps_h")
                nc.tensor.matmul(out=ph[:], lhsT=W1sb, rhs=ztT3[:],
                                 start=True, stop=True)
                th = wp.tile([SEM_H, P * 128], F32, tag="th")
                nc.scalar.activation(th[:], ph[:],
                                     mybir.ActivationFunctionType.Tanh,
                                     bias=b1sb)
                for p in range(P):
                    pw = ps1.tile([128, 1], F32, tag="ps_small")
                    nc.tensor.matmul(out=pw[:],
                                     lhsT=th[:, p * 128:(p + 1) * 128],
                                     rhs=w2sb, start=True, stop=True)
                    nc.vector.tensor_copy(
                        wbuf[:, p * NT + v:p * NT + v + 1], pw[:])

            # ---- semantic softmax over paths (global mean via AllReduce) ----
            wm = pp.tile([128, P * NT], F32)
            nc.vector.tensor_tensor(
                out=wm[:].rearrange("q (p v) -> q p v", p=P),
                in0=wbuf[:].rearrange("q (p v) -> q p v", p=P),
                in1=nmsk_t[:, None, :].broadcast_to([128, P, NT]),
                op=mybir.AluOpType.mult)
            ws3 = pp.tile([128, P], F32)
            nc.vector.reduce_sum(ws3[:, :, None],
                                 wm[:].rearrange("q (p v) -> q p v", p=P),
                                 axis=mybir.AxisListType.X)
            pt3 = ps1.tile([1, P], F32, tag="ps_small")
            nc.tensor.matmul(out=pt3[:], lhsT=onesc[:], rhs=ws3[:],
                             start=True, stop=True)
            sb4 = pp.tile([1, 4], F32)
            nc.gpsimd.memset(sb4[:], 0.0)
            nc.vector.tensor_copy(sb4[:, 0:P], pt3[:])
            nc.sync.dma_start(crin[:], sb4[:])
            nc.gpsimd.collective_compute(
                "AllReduce", mybir.AluOpType.add,
                replica_groups=[list(range(NC_))],
                ins=[crin[:]], outs=[crout[:]])
            ar4 = pp.tile([1, 4], F32)
            nc.sync.dma_start(ar4[:], crout[:])
            ex3 = pp.tile([1, P], F32)
            nc.scalar.activation(ex3[:], ar4[:, 0:P],
                                 mybir.ActivationFunctionType.Exp,
                                 scale=1.0 / N)
            ssum = pp.tile([1, 1], F32)
            nc.vector.reduce_sum(ssum[:], ex3[:], axis=mybir.AxisListType.X)
            rs = pp.tile([1, 1], F32)
            nc.vector.reciprocal(rs[:], ssum[:])
            beta = pp.tile([1, P], F32)
            nc.vector.tensor_tensor(out=beta[:], in0=ex3[:],
                                    in1=rs[:].broadcast_to([1, P]),
                                    op=mybir.AluOpType.mult)
            pb = ps1.tile([128, P], F32, tag="ps_small")
            nc.tensor.matmul(out=pb[:], lhsT=ones1[:], rhs=beta[:],
                             start=True, stop=True)
            betab = pp.tile([128, P], F32)
            nc.vector.tensor_copy(betab[:], pb[:])

            # ---- final combine + scatter to output rows ----
            for v in range(NT):
                cz = wp.tile([128, P, D], F32, tag="cz")
                nc.vector.tensor_tensor(
                    out=cz[:], in0=zbuf[:, v, :, :],
                    in1=betab[:, :, None].broadcast_to([128, P, D]),
                    op=mybir.AluOpType.mult)
                o = wp.tile([128, D], BF16, tag="o")
                with nc.allow_low_precision(reason="3-term combine to bf16"):
                    nc.vector.reduce_sum(o[:, :, None],
                                         cz[:].rearrange("p q d -> p d q"),
                                         axis=mybir.AxisListType.X)
                nc.gpsimd.indirect_dma_start(
                    out=out[:], out_offset=bass.IndirectOffsetOnAxis(
                        ap=row_t[:, v:v + 1], axis=0),
                    in_=o[:], in_offset=None,
                    bounds_check=NSH - 1, oob_is_err=False)
    nc.compile()
    STAGES["build"] = _time.perf_counter() - t0
    return nc


class _Plan:
    def __init__(self, nc):
        t0 = _time.perf_counter()
        b2j.install_neuronx_cc_hook()
        self.nc = nc
        pname = nc.partition_id_tensor.name if nc.partition_id_tensor else None
        self.in_names, self.out_names, out_avals = [], [], []
        for alloc in nc.m.functions[0].allocations:
            if not isinstance(alloc, mybir.MemoryLocationSet):
                continue
            name = alloc.memorylocations[0].name
            if alloc.kind == "ExternalInput":
                if name != pname:
                    self.in_names.append(name)
            elif alloc.kind == "ExternalOutput":
                self.out_names.append(name)
                out_avals.append(jax.core.ShapedArray(
                    tuple(alloc.tensor_shape), mybir.dt.np(alloc.dtype)))
        n_params = len(self.in_names)
        n_outs = len(out_avals)
        all_names = list(self.in_names) + self.out_names
        if pname is not None:
            all_names.append(pname)
        out_avals = tuple(out_avals)

        def _body(*args):
            operands = list(args)
            if pname is not None:
                operands.append(b2j.partition_id_tensor())
            return tuple(b2j._bass_exec_p.bind(
                *operands, out_avals=out_avals,
                in_names=tuple(all_names), out_names=tuple(self.out_names),
                lowering_input_output_aliases=(),
                sim_require_finite=True, sim_require_nnan=True, nc=nc))

        jf = jax.jit(
            shard_map(_body, mesh=_MESH,
                      in_specs=(PartitionSpec("core"),) * (n_params + n_outs),
                      out_specs=(PartitionSpec("core"),) * n_outs,
                      check_rep=False),
            donate_argnums=tuple(range(n_params, n_params + n_outs)),
            keep_unused=True)

        in_sds = []
        for alloc in nc.m.functions[0].allocations:
            if not isinstance(alloc, mybir.MemoryLocationSet):
                continue
            name = alloc.memorylocations[0].name
            if alloc.kind == "ExternalInput" and name != pname:
                shp = tuple(alloc.tensor_shape)
                in_sds.append(jax.ShapeDtypeStruct(
                    (NC_ * shp[0],) + shp[1:], mybir.dt.np(alloc.dtype),
                    sharding=_SHARD))
        self.zero_fns = []
        out_sds = []
        for av in out_avals:
            gshp = (NC_ * av.shape[0],) + tuple(av.shape[1:])
            self.zero_fns.append(jax.jit(
                lambda s=gshp, d=av.dtype: jnp.zeros(s, d),
                out_shardings=_SHARD))
            out_sds.append(jax.ShapeDtypeStruct(gshp, av.dtype,
                                                sharding=_SHARD))
        STAGES["plan_setup"] = _time.perf_counter() - t0
        t1 = _time.perf_counter()
        self.compiled = jf.lower(*in_sds, *out_sds).compile()
        # pre-create the donated output buffers (cheap on-device memsets)
        self.dev_zero = [fn() for fn in self.zero_fns]
        STAGES["plan_compile"] = _time.perf_counter() - t1

    def run(self, staged):
        t0 = _time.perf_counter()
        dev_in = [staged[nm] for nm in self.in_names]
        dz = self.dev_zero
        self.dev_zero = None
        if dz is None or any(z.is_deleted() for z in dz):
            dz = [fn() for fn in self.zero_fns]
        outs = self.compiled(*dev_in, *dz)
        res = {nm: np.asarray(o) for nm, o in zip(self.out_names, outs)}
        STAGES["exec"] = _time.perf_counter() - t0
        return res


_PLAN = None
_PLAN_BV = None
try:
    _PLAN = _Plan(_build(BV_DEFAULT))
    _PLAN_BV = list(BV_DEFAULT)
except Exception:
    import traceback
    traceback.print_exc()
    _PLAN = None


def kernel(h, src0, dst0, src1, dst1, src2, dst2, W, attn_l, attn_r,
           sem_W1, sem_b1, sem_w2):
    global LAST_WALL_NS, _PLAN, _PLAN_BV
    t_start = _time.perf_counter()
    h = np.asarray(h, np.float32)
    W = np.asarray(W, np.float32)
    attn_l = np.asarray(attn_l, np.float32)
    attn_r = np.asarray(attn_r, np.float32)
    srcs = [np.asarray(s, np.int64) for s in (src0, src1, src2)]
    dsts = [np.asarray(d, np.int64) for d in (dst0, dst1, dst2)]

    staged = {}

    # ---- hTk first so its (largest) transfer starts immediately ----
    hT = np.ascontiguousarray(h.T).astype(ml_dtypes.bfloat16)
    hTk = np.concatenate(
        [hT[:, k * NSH:(k + 1) * NSH] for k in range(NC_)], axis=0)
    staged["hTk"] = jax.device_put(hTk, _SHARD)

    # ---- small replicated tensors (packed) ----
    WpS = np.zeros((IN + P, P * 66), np.float32)
    for p in range(P):
        WpS[:IN, p * 66 + 0] = W[p] @ attn_r[p, 0]
        WpS[:IN, p * 66 + 1] = W[p] @ attn_l[p, 0]
        WpS[:IN, p * 66 + 2:p * 66 + 66] = W[p]
        WpS[IN + p, 1] = -300.0       # sentinel row content (cols 0..65)
    WS = np.zeros((SEM_H, SEM_H + 2), np.float32)
    WS[0:D, 0:SEM_H] = np.asarray(sem_W1, np.float32)
    WS[:, SEM_H] = np.asarray(sem_b1, np.float32)
    WS[:, SEM_H + 1] = np.asarray(sem_w2, np.float32)
    staged["WpS"] = jax.device_put(
        np.concatenate([WpS.astype(ml_dtypes.bfloat16)] * NC_, axis=0),
        _SHARD)
    staged["WS"] = jax.device_put(
        np.concatenate([WS] * NC_, axis=0), _SHARD)

    # ---- edge schedule ----
    t0 = _time.perf_counter()
    # degree-sorted lane permutation per core (over all paths)
    degall = np.zeros(N, np.int64)
    for p in range(P):
        degall += np.bincount(dsts[p], minlength=N)
    perms = [np.argsort(-degall[k * NSH:(k + 1) * NSH], kind="stable")
             for k in range(NC_)]
    # node -> global lane (core-major, degree-sorted within core)
    glane = np.empty(N, np.int64)
    lane_arange = np.arange(NSH)
    for k in range(NC_):
        glane[k * NSH + perms[k]] = k * NSH + lane_arange

    Bv = np.zeros(NT, np.int64)
    grids = []
    for p in range(P):
        lane = glane[dsts[p]]
        o = np.argsort(lane, kind="stable")
        lane_s, s_s = lane[o], srcs[p][o]
        starts = np.searchsorted(lane_s, np.arange(N))
        r = np.arange(len(lane_s)) - starts[lane_s]
        Bmax = int(r.max()) + 1 if len(r) else 1
        # padded grid rows: core k's lane l at k*NTL + l
        glp = lane_s + (lane_s // NSH) * (NTL - NSH)
        grid = np.full((NC_ * NTL, Bmax), SENTN, np.uint16)
        grid[glp, r] = s_s
        grids.append(grid)
        tl = glp // 128
        mx = np.full(NC_ * NT, -1, np.int64)
        np.maximum.at(mx, tl, r)
        Bv = np.maximum(Bv, (mx + 1).reshape(NC_, NT).max(axis=0))
    Bv = [int(x) for x in np.maximum(Bv, 1)]
    CT = int(sum(Bv)) * P

    coff = np.cumsum([0] + [P * b for b in Bv])
    srcM = np.full((NC_, 128, CT), SENTN, np.uint16)
    IDX = np.zeros((NC_, 128, 3 * NT), np.int32)
    for k in range(NC_):
        perm = perms[k]
        base = k * NTL
        for v in range(NT):
            b = Bv[v]
            rows = slice(base + v * 128, base + (v + 1) * 128)
            for p in range(P):
                take = min(b, grids[p].shape[1])
                c0 = int(coff[v]) + p * b
                srcM[k, :, c0:c0 + take] = grids[p][rows, :take]
            nn = min(128, NSH - v * 128)
            IDX[k, :, v] = 2 * N
            IDX[k, :nn, v] = perm[v * 128:v * 128 + nn]
            IDX[k, :nn, NT + v] = 1
            IDX[k, :, 2 * NT + v] = SENTN
            IDX[k, :nn, 2 * NT + v] = k * NSH + perm[v * 128:v * 128 + nn]
    STAGES["preprocess"] = _time.perf_counter() - t0

    staged["srcM"] = jax.device_put(srcM.reshape(NC_ * 128, CT), _SHARD)
    staged["IDX"] = jax.device_put(IDX.reshape(NC_ * 128, 3 * NT), _SHARD)

    if _PLAN is None or Bv != _PLAN_BV:
        _PLAN = _Plan(_build(Bv))
        _PLAN_BV = Bv
    results = _PLAN.run(staged)
    out = results["out"].astype(np.float32)

    LAST_WALL_NS = (_time.perf_counter() - t_start) * 1e9
    return out


# revision 7
# speedup vs baseline: 1.2831x; 1.2831x over previous
"""HAN layer (3-metapath GAT + semantic attention) on 8 TRN2 NeuronCores.

Sharding: nodes partitioned 6250/core; edges sharded by dst-node owner.
Each core projects only its LOCAL nodes (T_local = h_local @ Wp giving
[er|el|feat64] per metapath, paths interleaved so the full table row of
(node n, path p) is 3n+p), then an on-device AllGather builds the full
50k-node table on every core (~5x less host->device traffic than
shipping the full table per core). Per-core dst nodes are processed one
node per SBUF partition lane (degree-sorted for load balance); edges are
gathered per round with indirect DMA from the all-gathered table, the
metapath selected via the DMA element_offset. Padding uses sentinel
table rows (el=-300 => exp ~ 0, feat=0) so no masks are needed.
Aggregation runs on the vector engine (multiply + strided reduce), not
per-edge matmuls. Semantic attention uses a tiny AllReduce.

Host-side: the per-tile round schedule (BV) is deterministic for the
fixed problem instance, so the bass build + walrus compile + PJRT
executable (and the donated output buffers) are all prepared at import
time; kernel() itself only does numpy preprocessing, async device_put
(dispatched as soon as each array is ready) and the execution.
"""

import time as _time

import numpy as np
import ml_dtypes
import jax
import jax.numpy as jnp
from jax.sharding import Mesh, PartitionSpec, NamedSharding
from jax.experimental.shard_map import shard_map

import concourse.bass as bass
import concourse.tile as tile
import concourse.bass2jax as b2j
from concourse import bacc, mybir
from concourse.masks import make_identity

N = 50000
E = 800000
P = 3
IN = 256
D = 64
SEM_H = 128
NEG = 0.2
NC_ = 8
NSH = N // NC_            # 6250 nodes per core
NT = (NSH + 127) // 128   # 49 node tiles per core
NTL = NT * 128            # padded lanes per core
SENTN = N                 # sentinel node id (pad); table rows 3N..3N+2
RT = 3 * N + 8            # gathered table rows (3 sentinel rows + pad)
BF16 = mybir.dt.bfloat16
F32 = mybir.dt.float32
I32 = mybir.dt.int32
U16 = mybir.dt.uint16

# Per-tile gather round counts for the fixed problem instance
# (seed-0 edge lists; max over 8 cores and 3 paths of the per-128-lane
# incoming-degree maximum, lanes degree-sorted). Recomputed at runtime;
# if it ever differs the kernel is rebuilt on the fly.
BV_DEFAULT = [36, 34, 34, 32, 32, 33, 30, 32, 30, 30, 30, 29, 29, 29,
              29, 28, 28, 29, 28, 30, 30, 28, 27, 28, 27, 27, 27, 29,
              26, 28, 28, 26, 25, 28, 25, 26, 25, 25, 26, 26, 25, 25,
              27, 23, 25, 24, 24, 22, 19]

LAST_WALL_NS = 0.0
STAGES = {}

_DEVICES = jax.devices()[:NC_]
_MESH = Mesh(np.asarray(_DEVICES), ("core",))
_SHARD = NamedSharding(_MESH, PartitionSpec("core"))
try:
    _WARM = jax.device_put(np.zeros((NC_, 8), np.float32), _SHARD)
except Exception:
    _WARM = None


def _build(Bv):
    t0 = _time.perf_counter()
    Bmax = max(Bv)
    CT = int(sum(Bv)) * P
    nc = bacc.Bacc("TRN2", target_bir_lowering=False, debug=False)
    hTk = nc.dram_tensor("hTk", [IN, NSH], BF16, kind="ExternalInput").ap()
    # WpS rows 0..255: fused projection weights [er_w|el_w|feat_w] per
    # path; rows 256..258: the sentinel table rows (cols 0..65).
    WpS = nc.dram_tensor("WpS", [IN + P, P * 66], BF16,
                         kind="ExternalInput").ap()
    srcM = nc.dram_tensor("srcM", [128, CT], U16, kind="ExternalInput").ap()
    # IDX cols [0,NT): scatter row ids; [NT,2NT): lane validity 0/1;
    # [2NT,3NT): global node id of each lane (for the er gather).
    IDX = nc.dram_tensor("IDX", [128, 3 * NT], I32,
                         kind="ExternalInput").ap()
    # WS cols 0..127: sem_W1 (rows 0..63); col 128: sem_b1; col 129: sem_w2
    WS = nc.dram_tensor("WS", [SEM_H, SEM_H + 2], F32,
                        kind="ExternalInput").ap()
    out = nc.dram_tensor("out", [NSH, D], BF16, kind="ExternalOutput").ap()
    Tloc = nc.dram_tensor("Tloc", [NSH, P, 66], BF16).ap()
    Tg = nc.dram_tensor("Tg", [RT, 66], BF16, addr_space="Shared").ap()
    crin = nc.dram_tensor("crin", [1, 4], F32).ap()
    crout = nc.dram_tensor("crout", [1, 4], F32, addr_space="Shared").ap()

    with tile.TileContext(nc) as tc:
        with (
            tc.tile_pool(name="persist", bufs=1) as pp,
            tc.tile_pool(name="work", bufs=3) as wp,
            tc.tile_pool(name="gpool", bufs=2) as gp,
            tc.tile_pool(name="psA", bufs=2, space="PSUM") as psa,
            tc.tile_pool(name="psS", bufs=1, space="PSUM") as ps1,
        ):
            Wp0 = pp.tile([128, P * 66], BF16)
            Wp1 = pp.tile([128, P * 66], BF16)
            nc.sync.dma_start(Wp0[:], WpS[0:128, :])
            nc.sync.dma_start(Wp1[:], WpS[128:256, :])
            sl = pp.tile([P, 66], BF16)
            nc.sync.dma_start(sl[:], WpS[256:256 + P, 0:66])
            identF = pp.tile([128, 128], F32)
            make_identity(nc, identF[:])
            WSt = pp.tile([SEM_H, SEM_H + 2], F32)
            nc.sync.dma_start(WSt[:], WS[:])
            W1sb = WSt[0:D, 0:SEM_H]
            b1sb = WSt[:, SEM_H:SEM_H + 1]
            w2sb = WSt[:, SEM_H + 1:SEM_H + 2]
            idx_t = pp.tile([128, 3 * NT], I32)
            nc.sync.dma_start(idx_t[:], IDX[:])
            row_t = idx_t[:, 0:NT]
            nmsk_t = pp.tile([128, NT], F32)
            nc.vector.tensor_copy(nmsk_t[:], idx_t[:, NT:2 * NT])
            gid3 = pp.tile([128, NT], I32)
            nc.vector.tensor_scalar_mul(gid3[:], idx_t[:, 2 * NT:3 * NT], 3)
            zbuf = pp.tile([128, NT, P, D], F32)
            wbuf = pp.tile([128, P * NT], F32)
            onesc = pp.tile([128, 1], F32)
            nc.gpsimd.memset(onesc[:], 1.0)
            ones1 = pp.tile([1, 128], F32)
            nc.gpsimd.memset(ones1[:], 1.0)
            # edge gather rows: si_all = 3 * src_node_id (path selected
            # via the gather's element_offset)
            siu = wp.tile([128, CT], U16, tag="siu")
            nc.sync.dma_start(siu[:], srcM[:])
            si_all = pp.tile([128, CT], I32)
            nc.vector.tensor_copy(si_all[:], siu[:])
            nc.vector.tensor_scalar_mul(si_all[:], si_all[:], 3)

            # ---- Phase A: T_local = h_localT.T @ Wp  (49 tiles) ----
            GRP = 8
            t0_ = 0
            while t0_ < NT:
                gt = min(GRP, NT - t0_)
                ncols = min(NSH - t0_ * 128, gt * 128)
                h0 = wp.tile([128, GRP * 128], BF16, tag="h0")
                h1 = wp.tile([128, GRP * 128], BF16, tag="h1")
                nc.sync.dma_start(h0[:, :ncols],
                                  hTk[0:128, t0_ * 128:t0_ * 128 + ncols])
                nc.sync.dma_start(h1[:, :ncols],
                                  hTk[128:256, t0_ * 128:t0_ * 128 + ncols])
                for i in range(gt):
                    t = t0_ + i
                    w = min(128, NSH - t * 128)
                    pa = psa.tile([128, P * 66], F32, tag="pa")
                    nc.tensor.matmul(out=pa[:w, :],
                                     lhsT=h0[:, i * 128:i * 128 + w],
                                     rhs=Wp0[:], start=True, stop=False)
                    nc.tensor.matmul(out=pa[:w, :],
                                     lhsT=h1[:, i * 128:i * 128 + w],
                                     rhs=Wp1[:], start=False, stop=True)
                    stg = wp.tile([128, P * 66], BF16, tag="stg")
                    nc.vector.tensor_copy(stg[:w, :], pa[:w, :])
                    for p in range(P):
                        nc.sync.dma_start(
                            Tloc[t * 128:t * 128 + w, p, :],
                            stg[:w, p * 66:(p + 1) * 66])
                t0_ += gt

            # ---- AllGather the table; sentinel rows after it ----
            nc.gpsimd.collective_compute(
                "AllGather", mybir.AluOpType.bypass,
                replica_groups=[list(range(NC_))],
                ins=[Tloc[:]], outs=[Tg[0:3 * N, :]])
            nc.sync.dma_start(Tg[3 * N:3 * N + P, :], sl[:])

            # ---- Phase B: per node tile, 3 paths stacked ----
            coff = np.cumsum([0] + [P * b for b in Bv])
            for v in range(NT):
                B = Bv[v]
                c0 = int(coff[v])
                G = gp.tile([128, P, Bmax, 66], BF16, tag="G")
                for p in range(P):
                    for b in range(B):
                        c = c0 + p * B + b
                        nc.gpsimd.indirect_dma_start(
                            out=G[:, p, b, :], out_offset=None, in_=Tg[:],
                            in_offset=bass.IndirectOffsetOnAxis(
                                ap=si_all[:, c:c + 1], axis=0),
                            element_offset=p * 66)
                er3 = wp.tile([128, P], BF16, tag="er3")
                for p in range(P):
                    nc.gpsimd.indirect_dma_start(
                        out=er3[:, p:p + 1], out_offset=None, in_=Tg[:],
                        in_offset=bass.IndirectOffsetOnAxis(
                            ap=gid3[:, v:v + 1], axis=0),
                        element_offset=p * 66)
                # e = leaky(el + er); ex = exp(e)
                Ef = wp.tile([128, P, Bmax], F32, tag="Ef")
                nc.vector.tensor_tensor(
                    out=Ef[:, :, :B], in0=G[:, :, :B, 1],
                    in1=er3[:, :, None].broadcast_to([128, P, B]),
                    op=mybir.AluOpType.add)
                Lk = wp.tile([128, P, Bmax], F32, tag="Lk")
                nc.vector.tensor_scalar_mul(Lk[:, :, :B], Ef[:, :, :B], NEG)
                nc.vector.tensor_tensor(out=Ef[:, :, :B], in0=Ef[:, :, :B],
                                        in1=Lk[:, :, :B],
                                        op=mybir.AluOpType.max)
                EX = wp.tile([128, P, Bmax], BF16, tag="EX")
                nc.scalar.activation(EX[:, :, :B], Ef[:, :, :B],
                                     mybir.ActivationFunctionType.Exp)
                den = wp.tile([128, P], F32, tag="den")
                nc.vector.reduce_sum(den[:, :, None], EX[:, :, :B],
                                     axis=mybir.AxisListType.X)
                # weighted aggregation on DVE
                FW = gp.tile([128, P, Bmax, D], BF16, tag="FW")
                nc.vector.tensor_tensor(
                    out=FW[:, :, :B, :], in0=G[:, :, :B, 2:66],
                    in1=EX[:, :, :B, None].broadcast_to([128, P, B, D]),
                    op=mybir.AluOpType.mult)
                agg = wp.tile([128, P, D], F32, tag="agg")
                nc.vector.reduce_sum(
                    agg[:, :, :, None],
                    FW[:, :, :B, :].rearrange("p q b d -> p q d b"),
                    axis=mybir.AxisListType.X)
                nc.vector.tensor_scalar_max(den[:], den[:], 1e-9)
                rec = wp.tile([128, P], F32, tag="rec")
                nc.vector.reciprocal(rec[:], den[:])
                zt = wp.tile([128, P, D], F32, tag="zt")
                nc.vector.tensor_tensor(
                    out=zt[:], in0=agg[:],
                    in1=rec[:, :, None].broadcast_to([128, P, D]),
                    op=mybir.AluOpType.mult)
                # elu: max(x,0) + exp(min(x,0)) - 1
                t1 = wp.tile([128, P, D], F32, tag="t1")
                nc.vector.tensor_scalar_min(t1[:], zt[:], 0.0)
                t2 = wp.tile([128, P, D], F32, tag="t2")
                nc.scalar.activation(t2[:], t1[:],
                                     mybir.ActivationFunctionType.Exp)
                t3 = wp.tile([128, P, D], F32, tag="t3")
                nc.vector.tensor_scalar_max(t3[:], zt[:], 0.0)
                nc.vector.tensor_tensor(out=t2[:], in0=t2[:], in1=t3[:],
                                        op=mybir.AluOpType.add)
                nc.vector.tensor_scalar_add(zbuf[:, v, :, :], t2[:], -1.0)
                # semantic score w = tanh(z @ W1 + b1) @ w2 per path
                ztT3 = wp.tile([D, P * 128], F32, tag="ztT3")
                for p in range(P):
                    pt = ps1.tile([D, 128], F32, tag="ps_t")
                    nc.tensor.transpose(out=pt[:], in_=zbuf[:, v, p, :],
                                        identity=identF[:])
                    nc.vector.tensor_copy(ztT3[:, p * 128:(p + 1) * 128],
                                          pt[:])
                ph = ps1.tile([SEM_H, P * 128], F32, tag="ps_h")
                nc.tensor.matmul(out=ph[:], lhsT=W1sb, rhs=ztT3[:],
                                 start=True, stop=True)
                th = wp.tile([SEM_H, P * 128], F32, tag="th")
                nc.scalar.activation(th[:], ph[:],
                                     mybir.ActivationFunctionType.Tanh,
                                     bias=b1sb)
                for p in range(P):
                    pw = ps1.tile([128, 1], F32, tag="ps_small")
                    nc.tensor.matmul(out=pw[:],
                                     lhsT=th[:, p * 128:(p + 1) * 128],
                                     rhs=w2sb, start=True, stop=True)
                    nc.vector.tensor_copy(
                        wbuf[:, p * NT + v:p * NT + v + 1], pw[:])

            # ---- semantic softmax over paths (global mean via AllReduce) ----
            wm = pp.tile([128, P * NT], F32)
            nc.vector.tensor_tensor(
                out=wm[:].rearrange("q (p v) -> q p v", p=P),
                in0=wbuf[:].rearrange("q (p v) -> q p v", p=P),
                in1=nmsk_t[:, None, :].broadcast_to([128, P, NT]),
                op=mybir.AluOpType.mult)
            ws3 = pp.tile([128, P], F32)
            nc.vector.reduce_sum(ws3[:, :, None],
                                 wm[:].rearrange("q (p v) -> q p v", p=P),
                                 axis=mybir.AxisListType.X)
            pt3 = ps1.tile([1, P], F32, tag="ps_small")
            nc.tensor.matmul(out=pt3[:], lhsT=onesc[:], rhs=ws3[:],
                             start=True, stop=True)
            sb4 = pp.tile([1, 4], F32)
            nc.gpsimd.memset(sb4[:], 0.0)
            nc.vector.tensor_copy(sb4[:, 0:P], pt3[:])
            nc.sync.dma_start(crin[:], sb4[:])
            nc.gpsimd.collective_compute(
                "AllReduce", mybir.AluOpType.add,
                replica_groups=[list(range(NC_))],
                ins=[crin[:]], outs=[crout[:]])
            ar4 = pp.tile([1, 4], F32)
            nc.sync.dma_start(ar4[:], crout[:])
            ex3 = pp.tile([1, P], F32)
            nc.scalar.activation(ex3[:], ar4[:, 0:P],
                                 mybir.ActivationFunctionType.Exp,
                                 scale=1.0 / N)
            ssum = pp.tile([1, 1], F32)
            nc.vector.reduce_sum(ssum[:], ex3[:], axis=mybir.AxisListType.X)
            rs = pp.tile([1, 1], F32)
            nc.vector.reciprocal(rs[:], ssum[:])
            beta = pp.tile([1, P], F32)
            nc.vector.tensor_tensor(out=beta[:], in0=ex3[:],
                                    in1=rs[:].broadcast_to([1, P]),
                                    op=mybir.AluOpType.mult)
            pb = ps1.tile([128, P], F32, tag="ps_small")
            nc.tensor.matmul(out=pb[:], lhsT=ones1[:], rhs=beta[:],
                             start=True, stop=True)
            betab = pp.tile([128, P], F32)
            nc.vector.tensor_copy(betab[:], pb[:])

            # ---- final combine + scatter to output rows ----
            for v in range(NT):
                cz = wp.tile([128, P, D], F32, tag="cz")
                nc.vector.tensor_tensor(
                    out=cz[:], in0=zbuf[:, v, :, :],
                    in1=betab[:, :, None].broadcast_to([128, P, D]),
                    op=mybir.AluOpType.mult)
                o = wp.tile([128, D], BF16, tag="o")
                with nc.allow_low_precision(reason="3-term combine to bf16"):
                    nc.vector.reduce_sum(o[:, :, None],
                                         cz[:].rearrange("p q d -> p d q"),
                                         axis=mybir.AxisListType.X)
                nc.gpsimd.indirect_dma_start(
                    out=out[:], out_offset=bass.IndirectOffsetOnAxis(
                        ap=row_t[:, v:v + 1], axis=0),
                    in_=o[:], in_offset=None,
                    bounds_check=NSH - 1, oob_is_err=False)
    nc.compile()
    STAGES["build"] = _time.perf_counter() - t0
    return nc


class _Plan:
    def __init__(self, nc):
        t0 = _time.perf_counter()
        b2j.install_neuronx_cc_hook()
        self.nc = nc
        pname = nc.partition_id_tensor.name if nc.partition_id_tensor else None
        self.in_names, self.out_names, out_avals = [], [], []
        for alloc in nc.m.functions[0].allocations:
            if not isinstance(alloc, mybir.MemoryLocationSet):
                continue
            name = alloc.memorylocations[0].name
            if alloc.kind == "ExternalInput":
                if name != pname:
                    self.in_names.append(name)
            elif alloc.kind == "ExternalOutput":
                self.out_names.append(name)
                out_avals.append(jax.core.ShapedArray(
                    tuple(alloc.tensor_shape), mybir.dt.np(alloc.dtype)))
        n_params = len(self.in_names)
        n_outs = len(out_avals)
        all_names = list(self.in_names) + self.out_names
        if pname is not None:
            all_names.append(pname)
        out_avals = tuple(out_avals)

        def _body(*args):
            operands = list(args)
            if pname is not None:
                operands.append(b2j.partition_id_tensor())
            return tuple(b2j._bass_exec_p.bind(
                *operands, out_avals=out_avals,
                in_names=tuple(all_names), out_names=tuple(self.out_names),
                lowering_input_output_aliases=(),
                sim_require_finite=True, sim_require_nnan=True, nc=nc))

        jf = jax.jit(
            shard_map(_body, mesh=_MESH,
                      in_specs=(PartitionSpec("core"),) * (n_params + n_outs),
                      out_specs=(PartitionSpec("core"),) * n_outs,
                      check_rep=False),
            donate_argnums=tuple(range(n_params, n_params + n_outs)),
            keep_unused=True)

        in_sds = []
        for alloc in nc.m.functions[0].allocations:
            if not isinstance(alloc, mybir.MemoryLocationSet):
                continue
            name = alloc.memorylocations[0].name
            if alloc.kind == "ExternalInput" and name != pname:
                shp = tuple(alloc.tensor_shape)
                in_sds.append(jax.ShapeDtypeStruct(
                    (NC_ * shp[0],) + shp[1:], mybir.dt.np(alloc.dtype),
                    sharding=_SHARD))
        self.zero_fns = []
        out_sds = []
        for av in out_avals:
            gshp = (NC_ * av.shape[0],) + tuple(av.shape[1:])
            self.zero_fns.append(jax.jit(
                lambda s=gshp, d=av.dtype: jnp.zeros(s, d),
                out_shardings=_SHARD))
            out_sds.append(jax.ShapeDtypeStruct(gshp, av.dtype,
                                                sharding=_SHARD))
        STAGES["plan_setup"] = _time.perf_counter() - t0
        t1 = _time.perf_counter()
        self.compiled = jf.lower(*in_sds, *out_sds).compile()
        # pre-create the donated output buffers (cheap on-device memsets)
        self.dev_zero = [fn() for fn in self.zero_fns]
        STAGES["plan_compile"] = _time.perf_counter() - t1

    def run(self, staged):
        t0 = _time.perf_counter()
        dev_in = [staged[nm] for nm in self.in_names]
        dz = self.dev_zero
        self.dev_zero = None
        if dz is None or any(z.is_deleted() for z in dz):
            dz = [fn() for fn in self.zero_fns]
        outs = self.compiled(*dev_in, *dz)
        res = {nm: np.asarray(o) for nm, o in zip(self.out_names, outs)}
        STAGES["exec"] = _time.perf_counter() - t0
        return res


_PLAN = None
_PLAN_BV = None
try:
    _PLAN = _Plan(_build(BV_DEFAULT))
    _PLAN_BV = list(BV_DEFAULT)
except Exception:
    import traceback
    traceback.print_exc()
    _PLAN = None


def kernel(h, src0, dst0, src1, dst1, src2, dst2, W, attn_l, attn_r,
           sem_W1, sem_b1, sem_w2):
    global LAST_WALL_NS, _PLAN, _PLAN_BV
    t_start = _time.perf_counter()
    h = np.asarray(h, np.float32)
    W = np.asarray(W, np.float32)
    attn_l = np.asarray(attn_l, np.float32)
    attn_r = np.asarray(attn_r, np.float32)
    srcs = [np.asarray(s, np.int64) for s in (src0, src1, src2)]
    dsts = [np.asarray(d, np.int64) for d in (dst0, dst1, dst2)]

    staged = {}

    # ---- hTk first so its (largest) transfer starts immediately ----
    hT = np.ascontiguousarray(h.T).astype(ml_dtypes.bfloat16)
    hTk = np.concatenate(
        [hT[:, k * NSH:(k + 1) * NSH] for k in range(NC_)], axis=0)
    staged["hTk"] = jax.device_put(hTk, _SHARD)

    # ---- small replicated tensors (packed) ----
    WpS = np.zeros((IN + P, P * 66), np.float32)
    for p in range(P):
        WpS[:IN, p * 66 + 0] = W[p] @ attn_r[p, 0]
        WpS[:IN, p * 66 + 1] = W[p] @ attn_l[p, 0]
        WpS[:IN, p * 66 + 2:p * 66 + 66] = W[p]
        WpS[IN + p, 1] = -300.0       # sentinel row content (cols 0..65)
    WS = np.zeros((SEM_H, SEM_H + 2), np.float32)
    WS[0:D, 0:SEM_H] = np.asarray(sem_W1, np.float32)
    WS[:, SEM_H] = np.asarray(sem_b1, np.float32)
    WS[:, SEM_H + 1] = np.asarray(sem_w2, np.float32)
    staged["WpS"] = jax.device_put(
        np.concatenate([WpS.astype(ml_dtypes.bfloat16)] * NC_, axis=0),
        _SHARD)
    staged["WS"] = jax.device_put(
        np.concatenate([WS] * NC_, axis=0), _SHARD)

    # ---- edge schedule ----
    t0 = _time.perf_counter()
    # degree-sorted lane permutation per core (over all paths)
    degall = np.zeros(N, np.int64)
    for p in range(P):
        degall += np.bincount(dsts[p], minlength=N)
    perms = [np.argsort(-degall[k * NSH:(k + 1) * NSH], kind="stable")
             for k in range(NC_)]
    # node -> global lane (core-major, degree-sorted within core)
    glane = np.empty(N, np.int64)
    lane_arange = np.arange(NSH)
    for k in range(NC_):
        glane[k * NSH + perms[k]] = k * NSH + lane_arange

    Bv = np.zeros(NT, np.int64)
    grids = []
    for p in range(P):
        lane = glane[dsts[p]]
        o = np.argsort(lane, kind="stable")
        lane_s, s_s = lane[o], srcs[p][o]
        starts = np.searchsorted(lane_s, np.arange(N))
        r = np.arange(len(lane_s)) - starts[lane_s]
        Bmax = int(r.max()) + 1 if len(r) else 1
        # padded grid rows: core k's lane l at k*NTL + l
        glp = lane_s + (lane_s // NSH) * (NTL - NSH)
        grid = np.full((NC_ * NTL, Bmax), SENTN, np.uint16)
        grid[glp, r] = s_s
        grids.append(grid)
        tl = glp // 128
        mx = np.full(NC_ * NT, -1, np.int64)
        np.maximum.at(mx, tl, r)
        Bv = np.maximum(Bv, (mx + 1).reshape(NC_, NT).max(axis=0))
    Bv = [int(x) for x in np.maximum(Bv, 1)]
    CT = int(sum(Bv)) * P

    coff = np.cumsum([0] + [P * b for b in Bv])
    srcM = np.full((NC_, 128, CT), SENTN, np.uint16)
    IDX = np.zeros((NC_, 128, 3 * NT), np.int32)
    for k in range(NC_):
        perm = perms[k]
        base = k * NTL
        for v in range(NT):
            b = Bv[v]
            rows = slice(base + v * 128, base + (v + 1) * 128)
            for p in range(P):
                take = min(b, grids[p].shape[1])
                c0 = int(coff[v]) + p * b
                srcM[k, :, c0:c0 + take] = grids[p][rows, :take]
            nn = min(128, NSH - v * 128)
            IDX[k, :, v] = 2 * N
            IDX[k, :nn, v] = perm[v * 128:v * 128 + nn]
            IDX[k, :nn, NT + v] = 1
            IDX[k, :, 2 * NT + v] = SENTN
            IDX[k, :nn, 2 * NT + v] = k * NSH + perm[v * 128:v * 128 + nn]
    STAGES["preprocess"] = _time.perf_counter() - t0

    staged["srcM"] = jax.device_put(srcM.reshape(NC_ * 128, CT), _SHARD)
    staged["IDX"] = jax.device_put(IDX.reshape(NC_ * 128, 3 * NT), _SHARD)

    try:
        if _PLAN is None or Bv != _PLAN_BV:
            _PLAN = _Plan(_build(Bv))
            _PLAN_BV = Bv
        results = _PLAN.run(staged)
        out = results["out"].astype(np.float32)
    except Exception:
        import traceback
        traceback.print_exc()
        from concourse.bass_utils import run_bass_kernel_spmd
        nc = _build(Bv)
        in_maps = [{
            "hTk": hTk[k * IN:(k + 1) * IN],
            "WpS": WpS.astype(ml_dtypes.bfloat16), "WS": WS,
            "srcM": srcM[k], "IDX": IDX[k],
        } for k in range(NC_)]
        res = run_bass_kernel_spmd(nc, in_maps, core_ids=list(range(NC_)))
        out = np.concatenate(
            [res.results[k]["out"] for k in range(NC_)], axis=0
        ).astype(np.float32)

    LAST_WALL_NS = (_time.perf_counter() - t_start) * 1e9
    return out
